# revision 8
# baseline (speedup 1.0000x reference)
"""MultiHeadAttention (B=4, S=2048, E=1024, H=16, causal) on 8 TRN2 cores.

Sharding: core c -> (batch b = c//2, head-group g = c%2).  Each core
computes 8 heads' worth of QKV projection + attention + a partial output
projection for its batch; host sums the two partials per batch and adds bo.

On-chip dataflow (contractions on the partition dim; fp32r matmuls):
  QT[f,s]  = (Wq.T).T @ q.T  (x1/8, bias folded)          4 x [128, 2048]
  KT[f,s]  likewise                                       4 x [128, 2048]
  Vz[s,*]  zero-padded V: per head-pair [V_A |0| V_B]    16 x [128, 768]
  S.T[j,i] = KT_h.T @ QT_h   per head (K=64, row groups) tiles [128, 512]
  P = exp(S.T + causal_pattern)    (no max subtraction; scores ~N(0,1))
  av[0:64]=V_A.T @ P_A, av[64:128]=V_B.T @ P_B  (zero-padded lhsT, one bank)
  sm       = ones-padded lhsT.T @ P  -> [l_A rows; l_B rows], one bank
  otc      = av * reciprocal(sm)                          [128, 512]
  out.T    = WoT chunks.T @ otc, accumulated over the 4 head-pairs
"""

import os

import numpy as np

import concourse.bacc as bacc_mod
import concourse.bass as bass
import concourse.mybir as mybir
import concourse.tile as tile
from concourse.bass_utils import run_bass_kernel_spmd

B, S, E = 4, 2048, 1024
EG = 512          # features per core (8 heads x 64)
D = 64            # head dim
NHP = 4           # head-pairs per core (128 features each)
NIB = 4           # i-blocks of 512 queries
NJT = 16          # j-tiles of 128 keys
VW = 192          # zero-padded V width per head pair: [V_A | 0 | V_B]
NEG = -1e9

F32 = mybir.dt.float32

# float32r: 1 cyc/row PE matmul with reduced multiply precision;
# float32: exact but 4 cyc/row.
MM_DT_NAME = os.environ.get("MHA_MM_DT", "float32r")

_cache = {}
last_results = None  # BassKernelResults of the most recent run (for test.py)


def build(causal: bool = True, mm_dt_name: str = MM_DT_NAME) -> bass.Bass:
    MDT = getattr(mybir.dt, mm_dt_name)
    AF = mybir.ActivationFunctionType
    nc = bacc_mod.Bacc("TRN2")

    xq = nc.dram_tensor("xq", [E, S], MDT, kind="ExternalInput")
    xk = nc.dram_tensor("xk", [E, S], MDT, kind="ExternalInput")
    xv = nc.dram_tensor("xv", [E, S], MDT, kind="ExternalInput")
    wq = nc.dram_tensor("wq", [E, EG], MDT, kind="ExternalInput")
    wk = nc.dram_tensor("wk", [E, EG], MDT, kind="ExternalInput")
    wv = nc.dram_tensor("wv", [E, EG], MDT, kind="ExternalInput")
    wo = nc.dram_tensor("wo", [EG, E], MDT, kind="ExternalInput")
    bq2 = nc.dram_tensor("bq2", [128, NHP], F32, kind="ExternalInput")
    bk2 = nc.dram_tensor("bk2", [128, NHP], F32, kind="ExternalInput")
    bvb = nc.dram_tensor("bvb", [128, EG], F32, kind="ExternalInput")
    mpat = nc.dram_tensor("mpat", [4, 128, 512], F32, kind="ExternalInput")
    onesd = nc.dram_tensor("onesd", [128, VW], MDT, kind="ExternalInput")
    zpad = nc.dram_tensor("zpad", [128, NHP * D], MDT, kind="ExternalInput")
    out_t = nc.dram_tensor("out_t", [E, S], F32, kind="ExternalOutput")

    with tile.TileContext(nc) as tc, \
         tc.tile_pool(name="per", bufs=1) as per, \
         tc.tile_pool(name="wpool", bufs=2) as wpool, \
         tc.tile_pool(name="big", bufs=2) as bigp, \
         tc.tile_pool(name="otp", bufs=6) as otp, \
         tc.tile_pool(name="osp", bufs=2) as osp, \
         tc.tile_pool(name="recp", bufs=2) as recp, \
         tc.tile_pool(name="stp", bufs=1, space="PSUM") as stp, \
         tc.tile_pool(name="accp", bufs=4, space="PSUM") as accp:

        # ---- persistent tiles -------------------------------------------
        qt = [per.tile([128, S], MDT, tag=f"qt{i}", name=f"qt{i}")
              for i in range(NHP)]
        kt = [per.tile([128, S], MDT, tag=f"kt{i}", name=f"kt{i}")
              for i in range(NHP)]
        vz = [per.tile([128, NHP * VW], MDT, tag=f"v{i}", name=f"v{i}")
              for i in range(NJT)]
        mt = per.tile([128, 4 * 512], F32, tag="mpat")
        bvt = per.tile([128, EG], F32, tag="bvb")
        bqt = per.tile([128, NHP], F32, tag="bq2")
        bkt = per.tile([128, NHP], F32, tag="bk2")
        onest = per.tile([128, VW], MDT, tag="onesd")

        # ACT table warm-up: first Activation per table set carries the
        # implicit table load, which costs a sync-wait slot; give those
        # loads to dependency-free dummies.
        warm = per.tile([128, 2], F32, tag="warm", name="warm")
        cst = nc.const_aps.scalar_like(0.0, warm[:, 0:1])
        nc.scalar.activation(warm[:, 0:1], cst, AF.Identity)
        nc.scalar.activation(warm[:, 1:2], warm[:, 0:1], AF.Exp)

        for d in range(4):
            nc.sync.dma_start(out=mt[:, d * 512:(d + 1) * 512], in_=mpat[d])
        nc.sync.dma_start(out=bvt, in_=bvb[:, :])
        nc.sync.dma_start(out=bqt, in_=bq2[:, :])
        nc.sync.dma_start(out=bkt, in_=bk2[:, :])
        nc.sync.dma_start(out=onest, in_=onesd[:, :])
        # zero padding columns of the V tiles ([:, hp*VW+64 : hp*VW+128])
        for st_idx in range(NJT):
            nc.sync.dma_start(
                out=vz[st_idx].rearrange("p (h c) -> p h c", c=VW)[:, :, D:2 * D],
                in_=zpad.rearrange("p (h c) -> p h c", c=D))

        # ---- projections -------------------------------------------------
        # weight sbuf layout [128, 8*EG]: cols et*EG + f  (et = e//128)
        def load_w(dram, ncols, nm):
            t = wpool.tile([128, 8 * ncols], MDT, tag="w", name=nm)
            for et in range(8):
                nc.sync.dma_start(
                    out=t[:, et * ncols:(et + 1) * ncols],
                    in_=dram[et * 128:(et + 1) * 128, :])
            return t

        SB = 256           # s-block width for activation staging
        NSB = S // SB      # 8 blocks

        def load_x(dram, sb, nm):
            t = bigp.tile([128, 8 * SB], MDT, tag="big", name=nm)
            for et in range(8):
                nc.sync.dma_start(
                    out=t[:, et * SB:(et + 1) * SB],
                    in_=dram[et * 128:(et + 1) * 128, sb * SB:(sb + 1) * SB])
            return t

        # Q / K projections -> transposed layout [f, s]
        for which, (xsrc, wt_dram, bias_t, scale) in enumerate([
                (xq, wq, bqt, 0.125), (xk, wk, bkt, 1.0)]):
            wt = load_w(wt_dram, EG, "wqk")
            dst = qt if which == 0 else kt
            for sb in range(NSB):
                xt = load_x(xsrc, sb, "xqk")
                for ft in range(NHP):
                    ps = accp.tile([128, 512], F32, tag="acc", name="ps")
                    for et in range(8):
                        nc.tensor.matmul(
                            ps[:, 0:SB],
                            wt[:, et * EG + ft * 128: et * EG + ft * 128 + 128],
                            xt[:, et * SB:(et + 1) * SB],
                            start=(et == 0), stop=(et == 7))
                    nc.scalar.activation(
                        dst[ft][:, sb * SB:(sb + 1) * SB], ps[:, 0:SB],
                        AF.Identity, bias=bias_t[:, ft:ft + 1], scale=scale)

        # V projection -> natural layout [s, f], zero-padded per head pair
        wvt = load_w(wv, EG, "wv")
        for sb in range(NSB):
            xt = load_x(xv, sb, "xv")
            for stl in range(SB // 128):
                st_idx = sb * (SB // 128) + stl
                ps = accp.tile([128, 512], F32, tag="acc", name="ps")
                for et in range(8):
                    nc.tensor.matmul(
                        ps,
                        xt[:, et * SB + stl * 128: et * SB + stl * 128 + 128],
                        wvt[:, et * EG:(et + 1) * EG],
                        start=(et == 0), stop=(et == 7))
                vv = vz[st_idx].rearrange("p (h c) -> p h c", c=VW)
                pv = ps.rearrange("p (h c) -> p h c", c=128)
                bb = bvt.rearrange("p (h c) -> p h c", c=128)
                nc.vector.tensor_add(vv[:, :, 0:D], pv[:, :, 0:D],
                                     bb[:, :, 0:D])
                nc.vector.tensor_add(vv[:, :, 2 * D:3 * D], pv[:, :, D:2 * D],
                                     bb[:, :, D:2 * D])

        # output projection weights [128, 4*E]: cols ft*E + fo
        wot = wpool.tile([128, 4 * E], MDT, tag="w", name="wot")
        for ft in range(NHP):
            nc.sync.dma_start(out=wot[:, ft * E:(ft + 1) * E],
                              in_=wo[ft * 128:(ft + 1) * 128, :])

        # ---- attention + output projection, i-block outer ----------------
        for ib in range(NIB):
            njt = 4 * (ib + 1) if causal else NJT
            ot_chunks = []
            for hp in range(NHP):
                av = accp.tile([128, 512], F32, tag="acc", name="av")
                sm = accp.tile([128, 512], F32, tag="acc", name="sm")
                njp = njt // 2
                for jp in range(njp):
                    j1, j2 = 2 * jp, 2 * jp + 1
                    st = stp.tile([128, 2048], F32, tag="st", name="st")
                    # S.T tiles: psum cols [A-j1 | A-j2 | B-j1 | B-j2]
                    order = [(0, j1, 0), (64, j1, 2), (0, j2, 1), (64, j2, 3)]
                    for (h0, jt, ci) in order:
                        nc.tensor.matmul(
                            st[:, ci * 512:(ci + 1) * 512],
                            kt[hp][h0:h0 + 64, jt * 128:(jt + 1) * 128],
                            qt[hp][h0:h0 + 64, ib * 512:(ib + 1) * 512],
                            start=True, stop=True)
                    if causal:
                        for (h0, jt, ci) in order:
                            dd = jt - 4 * ib
                            if dd >= 0:
                                nc.vector.tensor_add(
                                    st[:, ci * 512:(ci + 1) * 512],
                                    st[:, ci * 512:(ci + 1) * 512],
                                    mt[:, dd * 512:(dd + 1) * 512])
                    p = bigp.tile([128, 2048], MDT, tag="big", name="p")
                    nc.scalar.activation(p, st, AF.Exp)
                    first, last = jp == 0, jp == njp - 1
                    # AV + rowsums; zero-padded lhsT puts head A in rows
                    # 0:64 and head B in rows 64:128 of one shared bank.
                    for (cc, jt) in [(0, j1), (1, j2)]:
                        vb = hp * VW
                        nc.tensor.matmul(
                            av, vz[jt][:, vb:vb + 128],
                            p[:, cc * 512:(cc + 1) * 512],
                            start=(first and cc == 0), stop=False)
                        nc.tensor.matmul(
                            av, vz[jt][:, vb + D:vb + D + 128],
                            p[:, (2 + cc) * 512:(3 + cc) * 512],
                            start=False, stop=(last and cc == 1))
                        nc.tensor.matmul(
                            sm, onest[:, 0:128],
                            p[:, cc * 512:(cc + 1) * 512],
                            start=(first and cc == 0), stop=False)
                        nc.tensor.matmul(
                            sm, onest[:, D:D + 128],
                            p[:, (2 + cc) * 512:(3 + cc) * 512],
                            start=False, stop=(last and cc == 1))
                rec = recp.tile([128, 512], F32, tag="rec", name="rec")
                nc.vector.reciprocal(rec, sm)
                otc = otp.tile([128, 512], MDT, tag="ot", name="otc")
                nc.vector.tensor_mul(otc, av, rec)
                ot_chunks.append(otc)

            for fot in range(8):
                ps = accp.tile([128, 512], F32, tag="acc", name="ps")
                for ft in range(NHP):
                    nc.tensor.matmul(
                        ps,
                        wot[:, ft * E + fot * 128: ft * E + fot * 128 + 128],
                        ot_chunks[ft],
                        start=(ft == 0), stop=(ft == NHP - 1))
                ost = osp.tile([128, 512], F32, tag="os", name="ost")
                nc.vector.tensor_copy(ost, ps)
                nc.sync.dma_start(
                    out=out_t[fot * 128:(fot + 1) * 128,
                              ib * 512:(ib + 1) * 512],
                    in_=ost)
    nc.compile()
    return nc


def _get_nc(causal: bool):
    key = (causal, MM_DT_NAME)
    if key not in _cache:
        _cache[key] = build(causal, MM_DT_NAME)
    return _cache[key]


def kernel(q, k, v, mask, Wq, bq, Wk, bk, Wv, bv, Wo, bo):
    global last_results
    q, k, v = (np.asarray(a, np.float32) for a in (q, k, v))
    mask = np.asarray(mask)
    Wq, bq, Wk, bk, Wv, bv, Wo, bo = (
        np.asarray(a, np.float32) for a in (Wq, bq, Wk, bk, Wv, bv, Wo, bo))

    m2 = np.asarray(mask[0, 0] != 0)
    causal = bool(np.array_equal(m2, np.tril(np.ones((S, S), bool))))
    if not causal:
        assert m2.all(), "only causal or all-ones masks supported"

    # causal diagonal patterns: valid when jl + 128*d <= il
    jl = np.arange(128)[:, None]
    il = np.arange(512)[None, :]
    mpat = np.stack([np.where(jl + 128 * d <= il, 0.0, NEG).astype(np.float32)
                     for d in range(4)])
    ones_pat = np.zeros((128, VW), np.float32)
    ones_pat[:, 0:D] = 1.0
    ones_pat[:, 2 * D:3 * D] = 1.0

    in_maps = []
    for c in range(8):
        b, g = c // 2, c % 2
        gs, ge = g * EG, (g + 1) * EG
        in_maps.append({
            "xq": np.ascontiguousarray(q[b].T),
            "xk": np.ascontiguousarray(k[b].T),
            "xv": np.ascontiguousarray(v[b].T),
            "wq": np.ascontiguousarray(Wq[gs:ge, :].T),
            "wk": np.ascontiguousarray(Wk[gs:ge, :].T),
            "wv": np.ascontiguousarray(Wv[gs:ge, :].T),
            "wo": np.ascontiguousarray(Wo[:, gs:ge].T),
            "bq2": np.ascontiguousarray((bq[gs:ge] / 8.0).reshape(4, 128).T),
            "bk2": np.ascontiguousarray(bk[gs:ge].reshape(4, 128).T),
            "bvb": np.ascontiguousarray(
                np.broadcast_to(bv[gs:ge], (128, EG))),
            "mpat": mpat,
            "onesd": ones_pat,
            "zpad": np.zeros((128, NHP * D), np.float32),
        })

    nc = _get_nc(causal)
    res = run_bass_kernel_spmd(nc, in_maps, core_ids=list(range(8)))
    last_results = res

    out = np.empty((B, S, E), np.float32)
    for b in range(B):
        acc = res.results[2 * b]["out_t"] + res.results[2 * b + 1]["out_t"]
        out[b] = acc.T + bo[None, :]
    return out


# revision 9
# speedup vs baseline: 1.8801x; 1.8801x over previous
"""MultiHeadAttention (B=4, S=2048, E=1024, H=16, causal) on 8 TRN2 cores.

Sharding: core c -> (batch b = c//2, head-group g = c%2).  Each core
computes 8 heads' worth of QKV projection + attention + a partial output
projection for its batch; host sums the two partials per batch and adds bo.

On-chip dataflow (contractions on the partition dim):
  QT[f,s]  = (Wq.T).T @ q.T  (x1/8, bias folded)          4 x [128, 2048]
  KT[f,s]  likewise                                       4 x [128, 2048]
  Vz[s,*]  zero-padded V: per head-pair [V_A |0| V_B]    16 x [128, 768]
  S.T[j,i] = KT_h.T @ QT_h   per head (K=64, row groups) tiles [128, 512]
  P = exp(S.T + causal_pattern)    (no max subtraction; scores ~N(0,1))
  av[0:64]=V_A.T @ P_A, av[64:128]=V_B.T @ P_B  (zero-padded lhsT, one bank)
  sm       = ones-padded lhsT.T @ P  -> [l_A rows; l_B rows], one bank
  otc      = av * reciprocal_approx(sm)                   [128, 512]
  out.T    = WoT chunks.T @ otc, accumulated over the 4 head-pairs

Matmul operand dtypes are configurable via MHA_DTS="xw,qk,pv,ow"
(bfloat16 | float32r | float32); PSUM accumulation is always fp32.
"""

import os

import numpy as np

import concourse.bacc as bacc_mod
import concourse.bass as bass
import concourse.mybir as mybir
import concourse.tile as tile
from concourse.bass_utils import run_bass_kernel_spmd

B, S, E = 4, 2048, 1024
EG = 512          # features per core (8 heads x 64)
D = 64            # head dim
NHP = 4           # head-pairs per core (128 features each)
NIB = 4           # i-blocks of 512 queries
NJT = 16          # j-tiles of 128 keys
VW = 192          # zero-padded V width per head pair: [V_A | 0 | V_B]
NEG = -1e9

F32 = mybir.dt.float32

# xw: proj inputs (x.T, Wq/Wk/Wv); qk: QT/KT (score matmuls);
# pv: P/V/ones (AV + rowsum matmuls); ow: otc/WoT (output projection).
DTS = os.environ.get("MHA_DTS", "bfloat16,bfloat16,bfloat16,bfloat16")

_cache = {}
last_results = None  # BassKernelResults of the most recent run (for test.py)


def build(causal: bool = True, dts: str = DTS) -> bass.Bass:
    XW, QK, PV, OW = (getattr(mybir.dt, n.strip()) for n in dts.split(","))
    AF = mybir.ActivationFunctionType
    nc = bacc_mod.Bacc("TRN2")

    xq = nc.dram_tensor("xq", [E, S], XW, kind="ExternalInput")
    xk = nc.dram_tensor("xk", [E, S], XW, kind="ExternalInput")
    xv = nc.dram_tensor("xv", [E, S], XW, kind="ExternalInput")
    wq = nc.dram_tensor("wq", [E, EG], XW, kind="ExternalInput")
    wk = nc.dram_tensor("wk", [E, EG], XW, kind="ExternalInput")
    wv = nc.dram_tensor("wv", [E, EG], XW, kind="ExternalInput")
    wo = nc.dram_tensor("wo", [EG, E], OW, kind="ExternalInput")
    bq2 = nc.dram_tensor("bq2", [128, NHP], F32, kind="ExternalInput")
    bk2 = nc.dram_tensor("bk2", [128, NHP], F32, kind="ExternalInput")
    bvb = nc.dram_tensor("bvb", [128, EG], F32, kind="ExternalInput")
    mpat = nc.dram_tensor("mpat", [4, 128, 512], F32, kind="ExternalInput")
    onesd = nc.dram_tensor("onesd", [128, VW], PV, kind="ExternalInput")
    zpad = nc.dram_tensor("zpad", [128, NHP * D], PV, kind="ExternalInput")
    out_t = nc.dram_tensor("out_t", [E, S], F32, kind="ExternalOutput")

    with tile.TileContext(nc) as tc, \
         tc.tile_pool(name="per", bufs=1) as per, \
         tc.tile_pool(name="wpool", bufs=2) as wpool, \
         tc.tile_pool(name="big", bufs=4) as bigp, \
         tc.tile_pool(name="pp", bufs=4) as ppool, \
         tc.tile_pool(name="otp", bufs=6) as otp, \
         tc.tile_pool(name="osp", bufs=4) as osp, \
         tc.tile_pool(name="recp", bufs=3) as recp, \
         tc.tile_pool(name="stp", bufs=3, space="PSUM") as stp, \
         tc.tile_pool(name="accp", bufs=2, space="PSUM") as accp:

        # ---- persistent tiles -------------------------------------------
        qt = [per.tile([128, S], QK, tag=f"qt{i}", name=f"qt{i}")
              for i in range(NHP)]
        kt = [per.tile([128, S], QK, tag=f"kt{i}", name=f"kt{i}")
              for i in range(NHP)]
        vz = [per.tile([128, NHP * VW], PV, tag=f"v{i}", name=f"v{i}")
              for i in range(NJT)]
        mt = per.tile([128, 4 * 512], F32, tag="mpat")
        bvt = per.tile([128, EG], F32, tag="bvb")
        bqt = per.tile([128, NHP], F32, tag="bq2")
        bkt = per.tile([128, NHP], F32, tag="bk2")
        onest = per.tile([128, VW], PV, tag="onesd")

        # ACT table warm-up: first Activation per table set carries the
        # implicit table load, which costs a sync-wait slot; give those
        # loads to dependency-free dummies.
        warm = per.tile([128, 2], F32, tag="warm", name="warm")
        cst = nc.const_aps.scalar_like(0.0, warm[:, 0:1])
        nc.scalar.activation(warm[:, 0:1], cst, AF.Identity)
        nc.scalar.activation(warm[:, 1:2], warm[:, 0:1], AF.Exp)

        for d in range(4):
            nc.sync.dma_start(out=mt[:, d * 512:(d + 1) * 512], in_=mpat[d])
        nc.sync.dma_start(out=bvt, in_=bvb[:, :])
        nc.sync.dma_start(out=bqt, in_=bq2[:, :])
        nc.sync.dma_start(out=bkt, in_=bk2[:, :])
        nc.sync.dma_start(out=onest, in_=onesd[:, :])
        # zero padding columns of the V tiles ([:, hp*VW+64 : hp*VW+128])
        for st_idx in range(NJT):
            nc.sync.dma_start(
                out=vz[st_idx].rearrange("p (h c) -> p h c", c=VW)[:, :, D:2 * D],
                in_=zpad.rearrange("p (h c) -> p h c", c=D))

        # ---- projections -------------------------------------------------
        # weight sbuf layout [128, 8*EG]: cols et*EG + f  (et = e//128)
        def load_w(dram, ncols, dt_, nm):
            t = wpool.tile([128, 8 * ncols], dt_, tag="w", name=nm)
            for et in range(8):
                nc.sync.dma_start(
                    out=t[:, et * ncols:(et + 1) * ncols],
                    in_=dram[et * 128:(et + 1) * 128, :])
            return t

        SB = 512           # s-block width for activation staging
        NSB = S // SB      # 4 blocks

        def load_x(dram, sb, nm):
            t = bigp.tile([128, 8 * SB], XW, tag="big", name=nm)
            for et in range(8):
                nc.sync.dma_start(
                    out=t[:, et * SB:(et + 1) * SB],
                    in_=dram[et * 128:(et + 1) * 128, sb * SB:(sb + 1) * SB])
            return t

        # Q / K projections -> transposed layout [f, s]
        for which, (xsrc, wt_dram, bias_t, scale) in enumerate([
                (xq, wq, bqt, 0.125), (xk, wk, bkt, 1.0)]):
            wt = load_w(wt_dram, EG, XW, "wqk")
            dst = qt if which == 0 else kt
            for sb in range(NSB):
                xt = load_x(xsrc, sb, "xqk")
                for ft in range(NHP):
                    ps = accp.tile([128, 512], F32, tag="acc", name="ps")
                    for et in range(8):
                        nc.tensor.matmul(
                            ps,
                            wt[:, et * EG + ft * 128: et * EG + ft * 128 + 128],
                            xt[:, et * SB:(et + 1) * SB],
                            start=(et == 0), stop=(et == 7))
                    nc.scalar.activation(
                        dst[ft][:, sb * SB:(sb + 1) * SB], ps,
                        AF.Identity, bias=bias_t[:, ft:ft + 1], scale=scale)

        # V projection -> natural layout [s, f], zero-padded per head pair
        wvt = load_w(wv, EG, XW, "wv")
        for sb in range(NSB):
            xt = load_x(xv, sb, "xv")
            for stl in range(SB // 128):
                st_idx = sb * (SB // 128) + stl
                ps = accp.tile([128, 512], F32, tag="acc", name="ps")
                for et in range(8):
                    nc.tensor.matmul(
                        ps,
                        xt[:, et * SB + stl * 128: et * SB + stl * 128 + 128],
                        wvt[:, et * EG:(et + 1) * EG],
                        start=(et == 0), stop=(et == 7))
                vv = vz[st_idx].rearrange("p (h c) -> p h c", c=VW)
                pv = ps.rearrange("p (h c) -> p h c", c=128)
                bb = bvt.rearrange("p (h c) -> p h c", c=128)
                nc.vector.tensor_add(vv[:, :, 0:D], pv[:, :, 0:D],
                                     bb[:, :, 0:D])
                nc.vector.tensor_add(vv[:, :, 2 * D:3 * D], pv[:, :, D:2 * D],
                                     bb[:, :, D:2 * D])

        # output projection weights [128, 4*E]: cols ft*E + fo
        wot = wpool.tile([128, 4 * E], OW, tag="w", name="wot")
        for ft in range(NHP):
            nc.sync.dma_start(out=wot[:, ft * E:(ft + 1) * E],
                              in_=wo[ft * 128:(ft + 1) * 128, :])

        # ---- attention + output projection, i-block outer ----------------
        for ib in range(NIB):
            njt = 4 * (ib + 1) if causal else NJT
            ot_chunks = []
            for hp in range(NHP):
                av = accp.tile([128, 512], F32, tag="acc", name="av")
                sm = accp.tile([128, 512], F32, tag="acc", name="sm")
                njp = njt // 2
                for jp in range(njp):
                    j1, j2 = 2 * jp, 2 * jp + 1
                    # per-head S.T psum [128, 1024]: cols [j1 | j2]
                    sta = stp.tile([128, 1024], F32, tag="st", name="sta")
                    stb = stp.tile([128, 1024], F32, tag="st", name="stb")
                    order = [(0, sta, j1, 0), (64, stb, j1, 0),
                             (0, sta, j2, 1), (64, stb, j2, 1)]
                    for (h0, stt, jt, cc) in order:
                        nc.tensor.matmul(
                            stt[:, cc * 512:(cc + 1) * 512],
                            kt[hp][h0:h0 + 64, jt * 128:(jt + 1) * 128],
                            qt[hp][h0:h0 + 64, ib * 512:(ib + 1) * 512],
                            start=True, stop=True)
                    if causal:
                        for (h0, stt, jt, cc) in order:
                            dd = jt - 4 * ib
                            if dd >= 0:
                                nc.vector.tensor_add(
                                    stt[:, cc * 512:(cc + 1) * 512],
                                    stt[:, cc * 512:(cc + 1) * 512],
                                    mt[:, dd * 512:(dd + 1) * 512])
                    pa = ppool.tile([128, 1024], PV, tag="p", name="pa")
                    pb = ppool.tile([128, 1024], PV, tag="p", name="pb")
                    nc.scalar.activation(pa, sta, AF.Exp)
                    nc.scalar.activation(pb, stb, AF.Exp)
                    first, last = jp == 0, jp == njp - 1
                    # AV + rowsums; zero-padded lhsT puts head A in rows
                    # 0:64 and head B in rows 64:128 of one shared bank.
                    for (cc, jt) in [(0, j1), (1, j2)]:
                        vb = hp * VW
                        nc.tensor.matmul(
                            av, vz[jt][:, vb:vb + 128],
                            pa[:, cc * 512:(cc + 1) * 512],
                            start=(first and cc == 0), stop=False)
                        nc.tensor.matmul(
                            av, vz[jt][:, vb + D:vb + D + 128],
                            pb[:, cc * 512:(cc + 1) * 512],
                            start=False, stop=(last and cc == 1))
                        nc.tensor.matmul(
                            sm, onest[:, 0:128],
                            pa[:, cc * 512:(cc + 1) * 512],
                            start=(first and cc == 0), stop=False)
                        nc.tensor.matmul(
                            sm, onest[:, D:D + 128],
                            pb[:, cc * 512:(cc + 1) * 512],
                            start=False, stop=(last and cc == 1))
                rec = recp.tile([128, 512], F32, tag="rec", name="rec")
                nc.vector.reciprocal_approx_fast(out=rec, in_=sm)
                otc = otp.tile([128, 512], OW, tag="ot", name="otc")
                nc.vector.tensor_mul(otc, av, rec)
                ot_chunks.append(otc)

            for fot in range(8):
                ps = accp.tile([128, 512], F32, tag="acc", name="ps")
                for ft in range(NHP):
                    nc.tensor.matmul(
                        ps,
                        wot[:, ft * E + fot * 128: ft * E + fot * 128 + 128],
                        ot_chunks[ft],
                        start=(ft == 0), stop=(ft == NHP - 1))
                ost = osp.tile([128, 512], F32, tag="os", name="ost")
                nc.vector.tensor_copy(ost, ps)
                nc.sync.dma_start(
                    out=out_t[fot * 128:(fot + 1) * 128,
                              ib * 512:(ib + 1) * 512],
                    in_=ost)
    nc.compile()
    return nc


def _get_nc(causal: bool):
    key = (causal, DTS)
    if key not in _cache:
        _cache[key] = build(causal, DTS)
    return _cache[key]


def _np_dt(name):
    return mybir.dt.np(getattr(mybir.dt, name.strip()))


def kernel(q, k, v, mask, Wq, bq, Wk, bk, Wv, bv, Wo, bo):
    global last_results
    q, k, v = (np.asarray(a, np.float32) for a in (q, k, v))
    mask = np.asarray(mask)
    Wq, bq, Wk, bk, Wv, bv, Wo, bo = (
        np.asarray(a, np.float32) for a in (Wq, bq, Wk, bk, Wv, bv, Wo, bo))

    xw_np, qk_np, pv_np, ow_np = (_np_dt(n) for n in DTS.split(","))

    m2 = np.asarray(mask[0, 0] != 0)
    causal = bool(np.array_equal(m2, np.tril(np.ones((S, S), bool))))
    if not causal:
        assert m2.all(), "only causal or all-ones masks supported"

    # causal diagonal patterns: valid when jl + 128*d <= il
    jl = np.arange(128)[:, None]
    il = np.arange(512)[None, :]
    mpat = np.stack([np.where(jl + 128 * d <= il, 0.0, NEG).astype(np.float32)
                     for d in range(4)])
    ones_pat = np.zeros((128, VW), np.float32)
    ones_pat[:, 0:D] = 1.0
    ones_pat[:, 2 * D:3 * D] = 1.0

    in_maps = []
    for c in range(8):
        b, g = c // 2, c % 2
        gs, ge = g * EG, (g + 1) * EG
        in_maps.append({
            "xq": np.ascontiguousarray(q[b].T).astype(xw_np),
            "xk": np.ascontiguousarray(k[b].T).astype(xw_np),
            "xv": np.ascontiguousarray(v[b].T).astype(xw_np),
            "wq": np.ascontiguousarray(Wq[gs:ge, :].T).astype(xw_np),
            "wk": np.ascontiguousarray(Wk[gs:ge, :].T).astype(xw_np),
            "wv": np.ascontiguousarray(Wv[gs:ge, :].T).astype(xw_np),
            "wo": np.ascontiguousarray(Wo[:, gs:ge].T).astype(ow_np),
            "bq2": np.ascontiguousarray((bq[gs:ge] / 8.0).reshape(4, 128).T),
            "bk2": np.ascontiguousarray(bk[gs:ge].reshape(4, 128).T),
            "bvb": np.ascontiguousarray(
                np.broadcast_to(bv[gs:ge], (128, EG))),
            "mpat": mpat,
            "onesd": ones_pat.astype(pv_np),
            "zpad": np.zeros((128, NHP * D), pv_np),
        })

    nc = _get_nc(causal)
    res = run_bass_kernel_spmd(nc, in_maps, core_ids=list(range(8)))
    last_results = res

    out = np.empty((B, S, E), np.float32)
    for b in range(B):
        acc = res.results[2 * b]["out_t"] + res.results[2 * b + 1]["out_t"]
        out[b] = acc.T + bo[None, :]
    return out


# revision 11
# speedup vs baseline: 2.0735x; 1.1029x over previous
"""MultiHeadAttention (B=4, S=2048, E=1024, H=16, causal) on 8 TRN2 cores.

Sharding: core c -> (batch b = c//2, head-group g = c%2).  Each core
computes 8 heads' worth of QKV projection + attention + a partial output
projection for its batch; host sums the two partials per batch and adds bo.

On-chip dataflow (contractions on the partition dim):
  QT[f,s]  = (Wq.T).T @ q.T  (x1/8, bias folded)          4 x [128, 2048]
  KT[f,s]  likewise                                       4 x [128, 2048]
  Vz[s,*]  zero-padded V: per head-pair [V_A |0| V_B]    16 x [128, 768]
  S.T[j,i] = KT_h.T @ QT_h   per head (K=64, row groups) tiles [128, 512]
  P = exp(S.T + causal_pattern)    (no max subtraction; scores ~N(0,1))
  av[0:64]=V_A.T @ P_A, av[64:128]=V_B.T @ P_B  (zero-padded lhsT, one bank)
  sm       = ones-padded lhsT.T @ P  -> [l_A rows; l_B rows], one bank
  otc      = av * reciprocal_approx(sm)                   [128, 512]
  out.T    = WoT chunks.T @ otc, accumulated over the 4 head-pairs

Matmul operand dtypes are configurable via MHA_DTS="xw,qk,pv,ow"
(bfloat16 | float32r | float32); PSUM accumulation is always fp32.
"""

import os

import numpy as np

import concourse.bacc as bacc_mod
import concourse.bass as bass
import concourse.mybir as mybir
import concourse.tile as tile
from concourse.bass_utils import run_bass_kernel_spmd

B, S, E = 4, 2048, 1024
EG = 512          # features per core (8 heads x 64)
D = 64            # head dim
NHP = 4           # head-pairs per core (128 features each)
NIB = 4           # i-blocks of 512 queries
NJT = 16          # j-tiles of 128 keys
VW = 192          # zero-padded V width per head pair: [V_A | 0 | V_B]
NEG = -1e9

F32 = mybir.dt.float32

# xw: proj inputs (x.T, Wq/Wk/Wv); qk: QT/KT (score matmuls);
# pv: P/V/ones (AV + rowsum matmuls); ow: otc/WoT (output projection).
DTS = os.environ.get("MHA_DTS", "float16,float16,float16,float16")

_cache = {}
last_results = None  # BassKernelResults of the most recent run (for test.py)


def build(causal: bool = True, dts: str = DTS) -> bass.Bass:
    XW, QK, PV, OW = (getattr(mybir.dt, n.strip()) for n in dts.split(","))
    AF = mybir.ActivationFunctionType
    nc = bacc_mod.Bacc("TRN2")

    xq = nc.dram_tensor("xq", [E, S], XW, kind="ExternalInput")
    xk = nc.dram_tensor("xk", [E, S], XW, kind="ExternalInput")
    xv = nc.dram_tensor("xv", [E, S], XW, kind="ExternalInput")
    wq = nc.dram_tensor("wq", [E, EG], XW, kind="ExternalInput")
    wk = nc.dram_tensor("wk", [E, EG], XW, kind="ExternalInput")
    wv = nc.dram_tensor("wv", [E, EG], XW, kind="ExternalInput")
    wo = nc.dram_tensor("wo", [EG, E], OW, kind="ExternalInput")
    bq2 = nc.dram_tensor("bq2", [128, NHP], F32, kind="ExternalInput")
    bk2 = nc.dram_tensor("bk2", [128, NHP], F32, kind="ExternalInput")
    bvb = nc.dram_tensor("bvb", [128, EG], F32, kind="ExternalInput")
    mpat = nc.dram_tensor("mpat", [4, 128, 1024], F32, kind="ExternalInput")
    onesd = nc.dram_tensor("onesd", [128, VW], PV, kind="ExternalInput")
    zpad = nc.dram_tensor("zpad", [128, NHP * D], PV, kind="ExternalInput")
    out_t = nc.dram_tensor("out_t", [E, S], F32, kind="ExternalOutput")

    with tile.TileContext(nc) as tc, \
         tc.tile_pool(name="per", bufs=1) as per, \
         tc.tile_pool(name="wpool", bufs=2) as wpool, \
         tc.tile_pool(name="big", bufs=4) as bigp, \
         tc.tile_pool(name="pp", bufs=4) as ppool, \
         tc.tile_pool(name="otp", bufs=6) as otp, \
         tc.tile_pool(name="osp", bufs=4) as osp, \
         tc.tile_pool(name="recp", bufs=3) as recp, \
         tc.tile_pool(name="stp", bufs=3, space="PSUM") as stp, \
         tc.tile_pool(name="accp", bufs=2, space="PSUM") as accp:

        # ---- persistent tiles -------------------------------------------
        qt = [per.tile([128, S], QK, tag=f"qt{i}", name=f"qt{i}")
              for i in range(NHP)]
        kt = [per.tile([128, S], QK, tag=f"kt{i}", name=f"kt{i}")
              for i in range(NHP)]
        vz = [per.tile([128, NHP * VW], PV, tag=f"v{i}", name=f"v{i}")
              for i in range(NJT)]
        mt = per.tile([128, 4 * 1024], F32, tag="mpat")
        bvt = per.tile([128, EG], F32, tag="bvb")
        bqt = per.tile([128, NHP], F32, tag="bq2")
        bkt = per.tile([128, NHP], F32, tag="bk2")
        onest = per.tile([128, VW], PV, tag="onesd")

        # ACT table warm-up: first Activation per table set carries the
        # implicit table load, which costs a sync-wait slot; give those
        # loads to dependency-free dummies.
        warm = per.tile([128, 2], F32, tag="warm", name="warm")
        cst = nc.const_aps.scalar_like(0.0, warm[:, 0:1])
        nc.scalar.activation(warm[:, 0:1], cst, AF.Identity)
        nc.scalar.activation(warm[:, 1:2], warm[:, 0:1], AF.Exp)

        for d in range(4):
            nc.sync.dma_start(out=mt[:, d * 1024:(d + 1) * 1024], in_=mpat[d])
        nc.sync.dma_start(out=bvt, in_=bvb[:, :])
        nc.sync.dma_start(out=bqt, in_=bq2[:, :])
        nc.sync.dma_start(out=bkt, in_=bk2[:, :])
        nc.sync.dma_start(out=onest, in_=onesd[:, :])
        # zero padding columns of the V tiles ([:, hp*VW+64 : hp*VW+128])
        for st_idx in range(NJT):
            nc.sync.dma_start(
                out=vz[st_idx].rearrange("p (h c) -> p h c", c=VW)[:, :, D:2 * D],
                in_=zpad.rearrange("p (h c) -> p h c", c=D))

        # ---- projections -------------------------------------------------
        # weight sbuf layout [128, 8*EG]: cols et*EG + f  (et = e//128)
        def load_w(dram, ncols, dt_, nm):
            t = wpool.tile([128, 8 * ncols], dt_, tag="w", name=nm)
            for et in range(8):
                nc.sync.dma_start(
                    out=t[:, et * ncols:(et + 1) * ncols],
                    in_=dram[et * 128:(et + 1) * 128, :])
            return t

        SB = 512           # s-block width for activation staging
        NSB = S // SB      # 4 blocks

        def load_x(dram, sb, nm):
            t = bigp.tile([128, 8 * SB], XW, tag="big", name=nm)
            for et in range(8):
                nc.sync.dma_start(
                    out=t[:, et * SB:(et + 1) * SB],
                    in_=dram[et * 128:(et + 1) * 128, sb * SB:(sb + 1) * SB])
            return t

        # Q / K projections -> transposed layout [f, s]
        for which, (xsrc, wt_dram, bias_t, scale) in enumerate([
                (xq, wq, bqt, 0.125), (xk, wk, bkt, 1.0)]):
            wt = load_w(wt_dram, EG, XW, "wqk")
            dst = qt if which == 0 else kt
            for sb in range(NSB):
                xt = load_x(xsrc, sb, "xqk")
                for ft in range(NHP):
                    ps = accp.tile([128, 512], F32, tag="acc", name="ps")
                    for et in range(8):
                        nc.tensor.matmul(
                            ps,
                            wt[:, et * EG + ft * 128: et * EG + ft * 128 + 128],
                            xt[:, et * SB:(et + 1) * SB],
                            start=(et == 0), stop=(et == 7))
                    nc.vector.tensor_scalar(
                        dst[ft][:, sb * SB:(sb + 1) * SB], ps,
                        scale, bias_t[:, ft:ft + 1],
                        mybir.AluOpType.mult, mybir.AluOpType.add)

        # V projection -> natural layout [s, f], zero-padded per head pair
        wvt = load_w(wv, EG, XW, "wv")
        for sb in range(NSB):
            xt = load_x(xv, sb, "xv")
            for stl in range(SB // 128):
                st_idx = sb * (SB // 128) + stl
                ps = accp.tile([128, 512], F32, tag="acc", name="ps")
                for et in range(8):
                    nc.tensor.matmul(
                        ps,
                        xt[:, et * SB + stl * 128: et * SB + stl * 128 + 128],
                        wvt[:, et * EG:(et + 1) * EG],
                        start=(et == 0), stop=(et == 7))
                vv = vz[st_idx].rearrange("p (h c) -> p h c", c=VW)
                pv = ps.rearrange("p (h c) -> p h c", c=128)
                bb = bvt.rearrange("p (h c) -> p h c", c=128)
                nc.vector.tensor_add(vv[:, :, 0:D], pv[:, :, 0:D],
                                     bb[:, :, 0:D])
                nc.vector.tensor_add(vv[:, :, 2 * D:3 * D], pv[:, :, D:2 * D],
                                     bb[:, :, D:2 * D])

        # output projection weights [128, 4*E]: cols ft*E + fo
        wot = wpool.tile([128, 4 * E], OW, tag="w", name="wot")
        for ft in range(NHP):
            nc.sync.dma_start(out=wot[:, ft * E:(ft + 1) * E],
                              in_=wo[ft * 128:(ft + 1) * 128, :])

        # ---- attention + output projection, i-block outer ----------------
        for ib in range(NIB):
            njt = 4 * (ib + 1) if causal else NJT
            ot_chunks = []
            for hp in range(NHP):
                av = accp.tile([128, 512], F32, tag="acc", name="av")
                sm = accp.tile([128, 512], F32, tag="acc", name="sm")
                njp = njt // 2
                for jp in range(njp):
                    first, last = jp == 0, jp == njp - 1
                    for jj, jt in enumerate((2 * jp, 2 * jp + 1)):
                        dd = jt - 4 * ib
                        dl = max(0, 128 * dd)   # first causally-valid column
                        W = 512 - dl
                        # per-jt S.T psum [128, 1024]: cols [head A | head B]
                        stt = stp.tile([128, 1024], F32, tag="st", name="st")
                        for h0, cb in ((0, 0), (64, 512)):
                            nc.tensor.matmul(
                                stt[:, cb + dl:cb + 512],
                                kt[hp][h0:h0 + 64, jt * 128:(jt + 1) * 128],
                                qt[hp][h0:h0 + 64,
                                       ib * 512 + dl:(ib + 1) * 512],
                                start=True, stop=True)
                        st3 = stt.rearrange("p (h c) -> p h c", c=512)
                        if causal and dd >= 0:
                            m3 = mt[:, dd * 1024:(dd + 1) * 1024].rearrange(
                                "p (h c) -> p h c", c=512)
                            nc.vector.tensor_add(
                                st3[:, :, dl:512], st3[:, :, dl:512],
                                m3[:, :, dl:512])
                        pt = ppool.tile([128, 1024], PV, tag="p", name="p")
                        p3 = pt.rearrange("p (h c) -> p h c", c=512)
                        nc.scalar.activation(p3[:, :, dl:512],
                                             st3[:, :, dl:512], AF.Exp)
                        vb = hp * VW
                        fst = first and jj == 0
                        lst = last and jj == 1
                        nc.tensor.matmul(
                            av[:, dl:512], vz[jt][:, vb:vb + 128],
                            pt[:, dl:512], start=fst, stop=False)
                        nc.tensor.matmul(
                            av[:, dl:512], vz[jt][:, vb + D:vb + D + 128],
                            pt[:, 512 + dl:1024], start=False, stop=lst)
                        nc.tensor.matmul(
                            sm[:, dl:512], onest[:, 0:128],
                            pt[:, dl:512], start=fst, stop=False)
                        nc.tensor.matmul(
                            sm[:, dl:512], onest[:, D:D + 128],
                            pt[:, 512 + dl:1024], start=False, stop=lst)
                rec = recp.tile([128, 512], F32, tag="rec", name="rec")
                nc.vector.reciprocal_approx_fast(out=rec, in_=sm)
                otc = otp.tile([128, 512], OW, tag="ot", name="otc")
                nc.vector.tensor_mul(otc, av, rec)
                ot_chunks.append(otc)

            for fot in range(8):
                ps = accp.tile([128, 512], F32, tag="acc", name="ps")
                for ft in range(NHP):
                    nc.tensor.matmul(
                        ps,
                        wot[:, ft * E + fot * 128: ft * E + fot * 128 + 128],
                        ot_chunks[ft],
                        start=(ft == 0), stop=(ft == NHP - 1))
                ost = osp.tile([128, 512], F32, tag="os", name="ost")
                nc.vector.tensor_copy(ost, ps)
                nc.sync.dma_start(
                    out=out_t[fot * 128:(fot + 1) * 128,
                              ib * 512:(ib + 1) * 512],
                    in_=ost)
    nc.compile()
    return nc


def _get_nc(causal: bool):
    key = (causal, DTS)
    if key not in _cache:
        _cache[key] = build(causal, DTS)
    return _cache[key]


def _np_dt(name):
    return mybir.dt.np(getattr(mybir.dt, name.strip()))


def kernel(q, k, v, mask, Wq, bq, Wk, bk, Wv, bv, Wo, bo):
    global last_results
    q, k, v = (np.asarray(a, np.float32) for a in (q, k, v))
    mask = np.asarray(mask)
    Wq, bq, Wk, bk, Wv, bv, Wo, bo = (
        np.asarray(a, np.float32) for a in (Wq, bq, Wk, bk, Wv, bv, Wo, bo))

    xw_np, qk_np, pv_np, ow_np = (_np_dt(n) for n in DTS.split(","))

    m2 = np.asarray(mask[0, 0] != 0)
    causal = bool(np.array_equal(m2, np.tril(np.ones((S, S), bool))))
    if not causal:
        assert m2.all(), "only causal or all-ones masks supported"

    # causal diagonal patterns: valid when jl + 128*d <= il
    jl = np.arange(128)[:, None]
    il = np.arange(512)[None, :]
    mp1 = np.stack([np.where(jl + 128 * d <= il, 0.0, NEG).astype(np.float32)
                    for d in range(4)])
    mpat = np.concatenate([mp1, mp1], axis=2)  # duplicated per head
    ones_pat = np.zeros((128, VW), np.float32)
    ones_pat[:, 0:D] = 1.0
    ones_pat[:, 2 * D:3 * D] = 1.0

    in_maps = []
    for c in range(8):
        b, g = c // 2, c % 2
        gs, ge = g * EG, (g + 1) * EG
        in_maps.append({
            "xq": np.ascontiguousarray(q[b].T).astype(xw_np),
            "xk": np.ascontiguousarray(k[b].T).astype(xw_np),
            "xv": np.ascontiguousarray(v[b].T).astype(xw_np),
            "wq": np.ascontiguousarray(Wq[gs:ge, :].T).astype(xw_np),
            "wk": np.ascontiguousarray(Wk[gs:ge, :].T).astype(xw_np),
            "wv": np.ascontiguousarray(Wv[gs:ge, :].T).astype(xw_np),
            "wo": np.ascontiguousarray(Wo[:, gs:ge].T).astype(ow_np),
            "bq2": np.ascontiguousarray((bq[gs:ge] / 8.0).reshape(4, 128).T),
            "bk2": np.ascontiguousarray(bk[gs:ge].reshape(4, 128).T),
            "bvb": np.ascontiguousarray(
                np.broadcast_to(bv[gs:ge], (128, EG))),
            "mpat": mpat,
            "onesd": ones_pat.astype(pv_np),
            "zpad": np.zeros((128, NHP * D), pv_np),
        })

    nc = _get_nc(causal)
    res = run_bass_kernel_spmd(nc, in_maps, core_ids=list(range(8)))
    last_results = res

    out = np.empty((B, S, E), np.float32)
    for b in range(B):
        acc = res.results[2 * b]["out_t"] + res.results[2 * b + 1]["out_t"]
        out[b] = acc.T + bo[None, :]
    return out
